# revision 18
# baseline (speedup 1.0000x reference)
"""MoE (top-2 of 8 experts) Trainium2 kernel, expert-parallel across 8 cores.

v3 (fully resident weights + pipelined):
  - 4 chunks of 1024 tokens, cap 320, slot groups (0,128,192) -- every W2
    matmul keeps 128 partitions; overlapped slots recompute (free).
  - W1 AND W2 both SBUF-resident (16 MB): the FFN stream never touches
    HBM, so ReduceScatter SDMA traffic cannot starve the Tensor engine.
  - gate: chunk 0 replicated (fp32 PE matmuls); tokens 1024..4095 gated
    data-parallel (each core gates its own 384 tokens; all-expert coefs
    AllGathered as [3072, 8] fp32). AG triggered before chunk-0
    compaction so it completes long before chunk-1 needs it.
  - phase-1 of chunk c+1 (slab, compaction, gather+PE-transpose) emitted
    after FFN(c); the Tile scheduler overlaps it with the FFN stream.
  - per-chunk bf16 ReduceScatter(add); out copies at the end.
"""

import numpy as np
import ml_dtypes

B, L, D, DFF, E = 2, 2048, 1024, 4096, 8
N = B * L                # 4096 tokens
P = 128
KD = D // P              # 8   contraction chunks over D
NDJ = DFF // P           # 32  DFF tiles
# (tk0, ntok, cap, slot-group offsets)
CHUNK_SPECS = [
    (0,    1024, 320, (0, 128, 192)),
    (1024, 1024, 320, (0, 128, 192)),
    (2048, 1024, 320, (0, 128, 192)),
    (3072, 768,  256, (0, 128)),
    (3840, 256,  128, (0,)),
]
NCHUNK = len(CHUNK_SPECS)
N_CORES = 8
OUT_OFFS = [0, 128, 256, 384, 480]   # per-rank output row offsets
OROWS = N // N_CORES                 # 512 output rows per rank
CAPMAX = 320
HALF = D // 2            # 512
DPTOK = 3072 // N_CORES  # 384 data-parallel-gated tokens per core
GCOL = 128               # gate matmul column tile (fp32 rhs)

_cache = {}


def _build():
    import concourse.bass as bass
    import concourse.mybir as mybir
    import concourse.tile as tile
    from concourse import bacc
    from concourse.masks import make_identity

    dt = mybir.dt
    AF = mybir.ActivationFunctionType
    OP = mybir.AluOpType

    nc = bacc.Bacc("TRN2", target_bir_lowering=False, debug=False,
                   num_devices=N_CORES)

    # ---- kernel I/O ----
    x_d = nc.dram_tensor("x", [N, D], dt.bfloat16, kind="ExternalInput")
    # gate inputs: cols 0..1023 = xT of chunk-0 tokens (replicated);
    # cols 1024..1407 = xT of this core's own 384 DP tokens.
    xtg_d = nc.dram_tensor("xtg", [P, KD, 1024 + DPTOK], dt.float32,
                           kind="ExternalInput")
    w1_d = nc.dram_tensor("w1", [P, NDJ, KD, P], dt.bfloat16,
                          kind="ExternalInput")
    w2_d = nc.dram_tensor("w2", [P, NDJ, D], dt.bfloat16,
                          kind="ExternalInput")
    b1_d = nc.dram_tensor("b1", [P, NDJ], dt.float32, kind="ExternalInput")
    b2_d = nc.dram_tensor("b2", [P, D], dt.bfloat16, kind="ExternalInput")
    wg_d = nc.dram_tensor("wg", [P, KD, E], dt.float32, kind="ExternalInput")
    bg_d = nc.dram_tensor("bg", [P, E], dt.float32, kind="ExternalInput")
    sel_d = nc.dram_tensor("sel", [P, E], dt.float32, kind="ExternalInput")
    lst_d = nc.dram_tensor("lst", [P, P], dt.float16, kind="ExternalInput")
    ust_d = nc.dram_tensor("ust", [16, 16], dt.float16, kind="ExternalInput")
    slot_d = nc.dram_tensor("slot", [P, CAPMAX], dt.float32,
                            kind="ExternalInput")
    iota_d = nc.dram_tensor("iota", [P, 1], dt.float32, kind="ExternalInput")

    out_d = nc.dram_tensor("out_shard", [OROWS, D], dt.bfloat16,
                           kind="ExternalOutput")

    rg = [list(range(N_CORES))]

    with tile.TileContext(nc) as tc:
        with (
            tc.tile_pool(name="const", bufs=1) as const,
            tc.tile_pool(name="xtgpool", bufs=2) as xtgpool,
            tc.tile_pool(name="xgpool", bufs=2) as xgpool,
            tc.tile_pool(name="xgtpool", bufs=2) as xgtpool,
            tc.tile_pool(name="hpool", bufs=1) as hpool,
            tc.tile_pool(name="ypool", bufs=4) as ypool,
            tc.tile_pool(name="ppool", bufs=2) as ppool,
            tc.tile_pool(name="spool", bufs=3) as spool,
            tc.tile_pool(name="chpool", bufs=2) as chpool,
            tc.tile_pool(name="psum", bufs=1, space="PSUM") as psum,
            tc.tile_pool(name="dram", bufs=1, space="DRAM") as dram,
        ):
            # ---------- constants ----------
            # gate-critical consts load first so gate matmuls start ASAP;
            # the rest stream behind them / on other queues.
            ident = const.tile([P, P], dt.float32, tag="ident")
            make_identity(nc, ident[:])
            identb = const.tile([P, P], dt.bfloat16, tag="identb")
            nc.vector.tensor_copy(identb[:], ident[:])
            identh = const.tile([P, P], dt.float16, tag="identh")
            nc.vector.tensor_copy(identh[:], ident[:])
            wgsb = const.tile([P, KD, E], dt.float32, tag="wgsb")
            nc.sync.dma_start(wgsb[:], wg_d[:])
            bgsb = const.tile([P, E], dt.float32, tag="bgsb")
            nc.sync.dma_start(bgsb[:], bg_d[:])
            selsb = const.tile([P, E], dt.float32, tag="selsb")
            lst = const.tile([P, P], dt.float16, tag="lst")
            ust = const.tile([16, 16], dt.float16, tag="ust")
            slotsb = const.tile([P, CAPMAX], dt.float32, tag="slotsb")
            iotasb = const.tile([P, 1], dt.float32, tag="iotasb")
            b1sb = const.tile([P, NDJ], dt.float32, tag="b1sb")
            b2b = const.tile([P, D], dt.bfloat16, tag="b2b")
            zt = const.tile([P, D], dt.bfloat16, tag="zt")
            nc.vector.memset(zt[:], 0.0)

            # resident weights: W1 on scalar queue, W2 on gpsimd queue
            # (both off the sync queue, which carries the gate loads).
            w1sb = const.tile([P, NDJ, KD, P], dt.bfloat16, tag="w1sb")
            for q in range(8):
                nc.scalar.dma_start(w1sb[:, 4 * q:4 * q + 4, :, :],
                                    w1_d[:, 4 * q:4 * q + 4, :, :])
            w2sb = const.tile([P, NDJ, D], dt.bfloat16, tag="w2sb")
            for q in range(8):
                nc.gpsimd.dma_start(w2sb[:, 4 * q:4 * q + 4, :],
                                    w2_d[:, 4 * q:4 * q + 4, :])

            def late_consts():
                nc.sync.dma_start(selsb[:], sel_d[:])
                nc.sync.dma_start(lst[:], lst_d[:])
                nc.sync.dma_start(ust[:], ust_d[:])
                nc.sync.dma_start(slotsb[:], slot_d[:])
                nc.sync.dma_start(iotasb[:], iota_d[:])
                nc.sync.dma_start(b1sb[:], b1_d[:])
                nc.sync.dma_start(b2b[:], b2_d[:])

            # internal DRAM: per-chunk partial + RS output; gate AG bufs
            partials = []
            rs_outs = []
            for c, (tk0, ntok, cap, sgo) in enumerate(CHUNK_SPECS):
                pc = dram.tile([ntok + 8, D], dt.bfloat16, tag=f"partial{c}")
                partials.append(pc)
                ro = dram.tile([ntok // N_CORES, D], dt.bfloat16,
                               tag=f"rsout{c}")
                rs_outs.append(ro)
            gatebuf = dram.tile([DPTOK // P, P, E], dt.float32,
                                tag="gatebuf")
            gateall = dram.tile([(DPTOK // P) * N_CORES, P, E], dt.float32,
                                tag="gateall")

            # =========== helpers ===========
            def gate(xcols, tpc, nm):
                """logit [P, tpc, E] = (xtg[:,:,xcols:+tpc*128]^T @ Wg) + bg.

                fp32 matmul chains of GCOL tokens, PE-transposed back."""
                logit = chpool.tile([P, tpc, E], dt.float32, tag="logit",
                                    name=f"logit{nm}")
                for col in range(0, tpc * P, GCOL):
                    pw = min(GCOL, tpc * P - col)
                    xtk = xtgpool.tile([P, KD, GCOL], dt.float32, tag="xtk",
                                       name=f"xtk{nm}_{col}")
                    nc.sync.dma_start(
                        xtk[:, :, :pw],
                        xtg_d[:, :, xcols + col:xcols + col + pw])
                    pgT = psum.tile([E, GCOL], dt.float32, tag="pa", bufs=2,
                                    name=f"pgT{nm}_{col}")
                    for kc in range(KD):
                        nc.tensor.matmul(pgT[:, :pw], lhsT=wgsb[:, kc, :],
                                         rhs=xtk[:, kc, :pw],
                                         start=(kc == 0), stop=(kc == KD - 1))
                    lgs = spool.tile([E, GCOL], dt.float32, tag="lgs",
                                     bufs=2)
                    nc.vector.tensor_copy(lgs[:, :pw], pgT[:, :pw])
                    for fo in range(pw // P):
                        f = (col + fo * P) // P
                        ptb = psum.tile([P, E], dt.float32, tag="pa",
                                        bufs=2, name=f"ptb{nm}_{f}")
                        nc.tensor.matmul(ptb[:],
                                         lhsT=lgs[:, fo * P:(fo + 1) * P],
                                         rhs=ident[:E, :E],
                                         is_transpose=True,
                                         start=True, stop=True)
                        nc.vector.tensor_add(logit[:, f, :], ptb[:], bgsb[:])
                return logit

            def softmax_own(logit, tpc, nm):
                """baseline top-2-of-8 for OUR expert: mask f16 + coef f32."""
                mask_ch = chpool.tile([P, tpc], dt.float16, tag="mask",
                                      name=f"mask{nm}")
                coef_ch = chpool.tile([P, tpc], dt.float32, tag="coef",
                                      name=f"coef{nm}")
                m1 = spool.tile([P, tpc], dt.float32, tag="m1")
                nc.vector.reduce_max(m1[:], logit[:],
                                     axis=mybir.AxisListType.X)
                eqm = spool.tile([P, tpc, E], dt.float32, tag="eqm")
                nc.vector.tensor_tensor(
                    eqm[:], logit[:],
                    m1[:, :, None].to_broadcast([P, tpc, E]), op=OP.is_ge)
                nc.vector.tensor_scalar_mul(eqm[:], eqm[:], 1e9)
                nc.vector.tensor_sub(eqm[:], logit[:], eqm[:])
                m2 = spool.tile([P, tpc], dt.float32, tag="m2")
                nc.vector.reduce_max(m2[:], eqm[:], axis=mybir.AxisListType.X)
                exps = spool.tile([P, tpc, E], dt.float32, tag="exps")
                nc.scalar.activation(exps[:], logit[:], AF.Exp)
                ssum = spool.tile([P, tpc], dt.float32, tag="ssum")
                nc.vector.reduce_sum(ssum[:], exps[:],
                                     axis=mybir.AxisListType.X)
                rinv = spool.tile([P, tpc], dt.float32, tag="rinv")
                nc.vector.reciprocal(rinv[:], ssum[:])
                selb = selsb[:, None, :].to_broadcast([P, tpc, E])
                tmp = spool.tile([P, tpc, E], dt.float32, tag="tmp")
                nc.vector.tensor_mul(tmp[:], logit[:], selb)
                lour = spool.tile([P, tpc], dt.float32, tag="lour")
                nc.vector.reduce_sum(lour[:], tmp[:],
                                     axis=mybir.AxisListType.X)
                nc.vector.tensor_mul(tmp[:], exps[:], selb)
                eour = spool.tile([P, tpc], dt.float32, tag="eour")
                nc.vector.reduce_sum(eour[:], tmp[:],
                                     axis=mybir.AxisListType.X)
                nc.vector.tensor_tensor(mask_ch[:], lour[:], m2[:],
                                        op=OP.is_ge)
                nc.vector.tensor_mul(coef_ch[:], eour[:], rinv[:])
                nc.vector.tensor_mul(coef_ch[:], coef_ch[:], mask_ch[:])
                return mask_ch, coef_ch

            def softmax_all(logit, tpc, nm):
                """all-expert top-2 coefs [P, tpc, E] (for the DP slice)."""
                coefE = spool.tile([P, tpc, E], dt.float32, tag="coefE",
                                   name=f"coefE{nm}", bufs=1)
                m1 = spool.tile([P, tpc], dt.float32, tag="m1")
                nc.vector.reduce_max(m1[:], logit[:],
                                     axis=mybir.AxisListType.X)
                eqm = spool.tile([P, tpc, E], dt.float32, tag="eqm")
                nc.vector.tensor_tensor(
                    eqm[:], logit[:],
                    m1[:, :, None].to_broadcast([P, tpc, E]), op=OP.is_ge)
                nc.vector.tensor_scalar_mul(eqm[:], eqm[:], 1e9)
                nc.vector.tensor_sub(eqm[:], logit[:], eqm[:])
                m2 = spool.tile([P, tpc], dt.float32, tag="m2")
                nc.vector.reduce_max(m2[:], eqm[:], axis=mybir.AxisListType.X)
                exps = spool.tile([P, tpc, E], dt.float32, tag="exps")
                nc.scalar.activation(exps[:], logit[:], AF.Exp)
                ssum = spool.tile([P, tpc], dt.float32, tag="ssum")
                nc.vector.reduce_sum(ssum[:], exps[:],
                                     axis=mybir.AxisListType.X)
                rinv = spool.tile([P, tpc], dt.float32, tag="rinv")
                nc.vector.reciprocal(rinv[:], ssum[:])
                nc.vector.tensor_tensor(
                    coefE[:], logit[:],
                    m2[:, :, None].to_broadcast([P, tpc, E]), op=OP.is_ge)
                nc.vector.tensor_mul(coefE[:], coefE[:], exps[:])
                nc.vector.tensor_mul(
                    coefE[:], coefE[:],
                    rinv[:, :, None].to_broadcast([P, tpc, E]))
                return coefE

            def compact(c, mask_t, coef_t):
                """stream-compact selected tokens; returns idx/coef tiles."""
                tk0, ntok, cap, sgo = CHUNK_SPECS[c]
                tpc = ntok // P
                nsg = len(sgo)
                mask_ch = mask_t[:, :tpc]
                coef_ch = coef_t[:, :tpc]
                mt_ps = psum.tile([16, P], dt.float16, tag="pc", bufs=1,
                                  name=f"mtps{c}")
                nc.tensor.matmul(mt_ps[:tpc, :], lhsT=mask_ch,
                                 rhs=identh[:], is_transpose=True,
                                 start=True, stop=True)
                mts = spool.tile([16, P], dt.float16, tag="mts")
                nc.vector.tensor_copy(mts[:tpc, :], mt_ps[:tpc, :])
                cs = spool.tile([16, 1], dt.float32, tag="cs")
                nc.vector.reduce_sum(cs[:tpc], mts[:tpc, :],
                                     axis=mybir.AxisListType.X)
                cs_b = spool.tile([16, P], dt.float16, tag="cs_b")
                nc.vector.tensor_copy(cs_b[:tpc],
                                      cs[:tpc].to_broadcast([tpc, P]))
                ppos = psum.tile([P, 16], dt.float32, tag="pc", bufs=1,
                                 name=f"ppos{c}")
                nc.tensor.matmul(ppos[:, :tpc], lhsT=lst[:], rhs=mask_ch,
                                 start=True, stop=False)
                nc.tensor.matmul(ppos[:, :tpc], lhsT=cs_b[:tpc],
                                 rhs=ust[:tpc, :tpc],
                                 start=False, stop=True)
                t1 = spool.tile([P, 16], dt.float32, tag="t1")
                nc.vector.tensor_scalar_add(t1[:, :tpc], ppos[:, :tpc],
                                            -float(cap))
                t2 = spool.tile([P, 16], dt.float32, tag="t2")
                nc.vector.tensor_mul(t2[:, :tpc], t1[:, :tpc], mask_ch)
                pos_eff = chpool.tile([P, 16], dt.float32, tag="pos_eff",
                                      name=f"pos_eff{c}")
                nc.vector.tensor_scalar_add(pos_eff[:, :tpc], t2[:, :tpc],
                                            float(cap))

                pcmp = psum.tile([P, 3 * 3], dt.float32, tag="pc",
                                 bufs=1, name=f"pcmp{c}")
                for f in range(tpc):
                    perm = ppool.tile([P, CAPMAX], dt.float16, tag="perm",
                                      name=f"perm{c}_{f}")
                    nc.vector.tensor_tensor(
                        perm[:, :cap],
                        pos_eff[:, f:f + 1].to_broadcast([P, cap]),
                        slotsb[:, :cap], op=OP.is_equal)
                    rhs3 = spool.tile([P, 3], dt.float16, tag="rhs3")
                    nc.vector.tensor_scalar_add(rhs3[:, 0:1], iotasb[:],
                                                float(f * P))
                    nc.vector.tensor_copy(rhs3[:, 1:2], coef_ch[:, f:f + 1])
                    nc.vector.memset(rhs3[:, 2:3], 1.0)
                    for sg in range(nsg):
                        nc.tensor.matmul(
                            pcmp[:, 3 * sg:3 * sg + 3],
                            lhsT=perm[:, sgo[sg]:sgo[sg] + P],
                            rhs=rhs3[:],
                            start=(f == 0 and sg == 0),
                            stop=(f == tpc - 1 and sg == nsg - 1))

                idx_g_i = chpool.tile([P, 3], dt.int32, tag="idx_g",
                                      name=f"idxg{c}")
                idx_s_i = chpool.tile([P, 3], dt.int32, tag="idx_s",
                                      name=f"idxs{c}")
                coef_sg = chpool.tile([P, 3], dt.float32, tag="coef_sg",
                                      name=f"coefsg{c}")
                for sg in range(nsg):
                    cmp = spool.tile([P, 3], dt.float32, tag="cmp")
                    nc.vector.tensor_copy(cmp[:], pcmp[:, 3 * sg:3 * sg + 3])
                    nc.vector.tensor_copy(coef_sg[:, sg:sg + 1], cmp[:, 1:2])
                    gidx = spool.tile([P, 1], dt.float32, tag="gidx")
                    nc.vector.tensor_scalar_add(gidx[:], cmp[:, 0:1],
                                                float(tk0))
                    nc.vector.tensor_copy(idx_g_i[:, sg:sg + 1], gidx[:])
                    iv = spool.tile([P, 1], dt.float32, tag="iv")
                    nc.vector.tensor_scalar(iv[:], cmp[:, 2:3],
                                            -float(ntok), float(ntok),
                                            op0=OP.mult, op1=OP.add)
                    sidx = spool.tile([P, 1], dt.float32, tag="sidx")
                    nc.vector.tensor_add(sidx[:], cmp[:, 0:1], iv[:])
                    nc.vector.tensor_copy(idx_s_i[:, sg:sg + 1], sidx[:])
                return idx_g_i, idx_s_i, coef_sg

            def gather(c, idx_g_i):
                """indirect-gather selected bf16 rows + PE transpose."""
                tk0, ntok, cap, sgo = CHUNK_SPECS[c]
                nsg = len(sgo)
                xgT = xgtpool.tile([P, KD, CAPMAX], dt.bfloat16, tag="xgT",
                                   name=f"xgT{c}")
                xgs = []
                for sg in range(nsg):
                    xg = xgpool.tile([P, D], dt.bfloat16, tag="xg", bufs=4,
                                     name=f"xg{c}_{sg}")
                    nc.gpsimd.indirect_dma_start(
                        out=xg[:], out_offset=None, in_=x_d[:, :],
                        in_offset=bass.IndirectOffsetOnAxis(
                            ap=idx_g_i[:, sg:sg + 1], axis=0))
                    xgs.append(xg)
                for sg in range(nsg):
                    for g in range(KD // 2):
                        pt2 = psum.tile([P, 2, P], dt.bfloat16, tag="pa",
                                        bufs=2, name=f"pt2_{c}_{sg}_{g}")
                        for j in range(2):
                            kc = 2 * g + j
                            nc.tensor.matmul(
                                pt2[:, j, :],
                                lhsT=xgs[sg][:, kc * P:(kc + 1) * P],
                                rhs=identb[:], is_transpose=True,
                                start=(j == 0), stop=(j == 1))
                        nc.vector.tensor_copy(
                            xgT[:, 2 * g:2 * g + 2, sgo[sg]:sgo[sg] + P],
                            pt2[:])
                return xgT

            def zero_partial(c):
                tk0, ntok, cap, sgo = CHUNK_SPECS[c]
                for i in range(ntok // P):
                    nc.sync.dma_start(
                        partials[c][i * P:(i + 1) * P, :], zt[:])

            def ffn_w1(c, xgT):
                tk0, ntok, cap, sgo = CHUNK_SPECS[c]
                hT = hpool.tile([P, NDJ, CAPMAX], dt.bfloat16, tag="hT",
                                name=f"hT{c}")
                for dj in range(NDJ):
                    ph = psum.tile([P, 320], dt.float32, tag="ph", bufs=2,
                                   name=f"ph{c}_{dj}")
                    for kc in range(KD):
                        nc.tensor.matmul(
                            ph[:, :cap],
                            lhsT=w1sb[:, dj, kc, :],
                            rhs=xgT[:, kc, :cap],
                            start=(kc == 0), stop=(kc == KD - 1))
                    nc.scalar.activation(hT[:, dj, :cap], ph[:, :cap],
                                         AF.Relu, bias=b1sb[:, dj:dj + 1])
                return hT

            def ffn_w2(c, hT, idx_s_i, coef_sg):
                tk0, ntok, cap, sgo = CHUNK_SPECS[c]
                nsg = len(sgo)
                youts = [ypool.tile([P, D], dt.bfloat16, tag="yout",
                                    name=f"yout{c}_{i}") for i in range(nsg)]
                for sg in range(nsg):
                    for h in range(2):
                        hs = slice(h * HALF, (h + 1) * HALF)
                        py = psum.tile([P, HALF], dt.float32, tag="pys",
                                       bufs=3, name=f"py{c}_{sg}_{h}")
                        for dj in range(NDJ):
                            nc.tensor.matmul(
                                py[:],
                                lhsT=hT[:, dj, sgo[sg]:sgo[sg] + P],
                                rhs=w2sb[:, dj, hs],
                                start=(dj == 0), stop=(dj == NDJ - 1))
                        ytmp = spool.tile([P, HALF], dt.float32, tag="ytmp",
                                          bufs=2)
                        nc.vector.tensor_add(ytmp[:], py[:], b2b[:, hs])
                        nc.vector.tensor_scalar_mul(youts[sg][:, hs],
                                                    ytmp[:],
                                                    coef_sg[:, sg:sg + 1])
                for sg in range(nsg):
                    nc.gpsimd.indirect_dma_start(
                        out=partials[c][:, :],
                        out_offset=bass.IndirectOffsetOnAxis(
                            ap=idx_s_i[:, sg:sg + 1], axis=0),
                        in_=youts[sg][:], in_offset=None)
                nc.gpsimd.collective_compute(
                    "ReduceScatter", mybir.AluOpType.add, replica_groups=rg,
                    ins=[partials[c][0:ntok, :].opt()],
                    outs=[rs_outs[c][:, :].opt()])

            def phase1_dp(c):
                """slab load + mask/coef + compact + gather for chunk c."""
                tk0, ntok, cap, sgo = CHUNK_SPECS[c]
                tpc = ntok // P
                slab = spool.tile([P, 8, E], dt.float32, tag="slab",
                                  name=f"slab{c}", bufs=2)
                f0 = (tk0 - 1024) // P
                # gpsimd queue: ordered behind the AllGather trigger, so
                # the collective-completion wait is enforced by queue order
                # (cross-queue sem waits on collectives proved racy).
                for f in range(tpc):
                    nc.gpsimd.dma_start(slab[:, f, :], gateall[f0 + f, :, :])
                selb = selsb[:, None, :].to_broadcast([P, tpc, E])
                tmp = spool.tile([P, 8, E], dt.float32, tag="tmpsl")
                nc.vector.tensor_mul(tmp[:, :tpc, :], slab[:, :tpc, :], selb)
                coef_ch = chpool.tile([P, 8], dt.float32, tag="coef",
                                      name=f"coef{c}")
                nc.vector.reduce_sum(coef_ch[:, :tpc], tmp[:, :tpc, :],
                                     axis=mybir.AxisListType.X)
                mask_ch = chpool.tile([P, 8], dt.float16, tag="mask",
                                      name=f"mask{c}")
                nc.vector.tensor_scalar(mask_ch[:, :tpc], coef_ch[:, :tpc],
                                        0.0, None, op0=OP.is_gt)
                idx_g_i, idx_s_i, coef_sg = compact(c, mask_ch, coef_ch)
                zero_partial(c)
                xgT = gather(c, idx_g_i)
                return xgT, idx_s_i, coef_sg

            # ============ emission ============
            # --- DP gate slice first (AG triggers early, completes in the
            # shadow of chunk-0's replicated gate + FFN) ---
            logitD = gate(1024, DPTOK // P, "dp")
            coefE = softmax_all(logitD, DPTOK // P, "dp")
            for f in range(DPTOK // P):
                nc.sync.dma_start(gatebuf[f, :, :], coefE[:, f, :])
            nc.gpsimd.collective_compute(
                "AllGather", mybir.AluOpType.bypass, replica_groups=rg,
                ins=[gatebuf[:, :, :].opt()],
                outs=[gateall[:, :, :].opt()])

            # --- chunk 0: replicated gate + compact + gather ---
            logit0 = gate(0, 8, "c0")
            late_consts()
            mask0, coef0 = softmax_own(logit0, 8, "c0")
            idx_g0, idx_s0, coef_sg0 = compact(0, mask0, coef0)
            zero_partial(0)
            xgT0 = gather(0, idx_g0)

            # --- pipelined FFN ---
            prev = (xgT0, idx_s0, coef_sg0)
            for c in range(NCHUNK):
                xgT, idx_s_i, coef_sg = prev
                hT = ffn_w1(c, xgT)
                ffn_w2(c, hT, idx_s_i, coef_sg)
                if c + 1 < NCHUNK:
                    prev = phase1_dp(c + 1)

            # out copies (earlier RS long done; last copy is the tail)
            for c, (tk0, ntok, cap, sgo) in enumerate(CHUNK_SPECS):
                nrr = ntok // N_CORES
                nc.gpsimd.dma_start(
                    out_d[OUT_OFFS[c]:OUT_OFFS[c] + nrr, :], rs_outs[c][:, :])

    nc.compile()
    return nc


def _host_inputs(x, W1, b1, W2, b2, Wg, bg):
    bf16 = ml_dtypes.bfloat16
    f32 = np.float32
    f16 = np.float16
    x2 = np.ascontiguousarray(x.reshape(N, D), dtype=f32)
    xb = x2.astype(bf16)
    xt = np.ascontiguousarray(x2.T)           # [D, N]
    xtt = xt.reshape(KD, P, N)                # [kc, p, tok]
    lst = np.triu(np.ones((P, P), f16), k=1)
    ust = np.triu(np.ones((16, 16), f16), k=1)
    slot = np.tile(np.arange(CAPMAX, dtype=f32), (P, 1))
    iota = np.arange(P, dtype=f32).reshape(P, 1)
    wg = np.ascontiguousarray(
        Wg.reshape(KD, P, E).transpose(1, 0, 2)).astype(f32)
    in_maps = []
    for e in range(N_CORES):
        sel = np.zeros((E,), f32)
        sel[e] = 1.0
        xtg = np.empty((P, KD, 1024 + DPTOK), f32)
        xtg[:, :, :1024] = xtt[:, :, :1024].transpose(1, 0, 2)
        t0 = 1024 + e * DPTOK
        xtg[:, :, 1024:] = xtt[:, :, t0:t0 + DPTOK].transpose(1, 0, 2)
        in_maps.append({
            "x": xb,
            "xtg": xtg,
            "w1": np.ascontiguousarray(
                W1[e].reshape(KD, P, NDJ, P).transpose(1, 2, 0, 3)
            ).astype(bf16),
            "w2": np.ascontiguousarray(
                W2[e].reshape(NDJ, P, D).transpose(1, 0, 2)).astype(bf16),
            "b1": np.ascontiguousarray(
                b1[e].reshape(NDJ, P).T).astype(f32),
            "b2": np.tile(b2[e].astype(bf16), (P, 1)),
            "wg": wg,
            "bg": np.tile(bg.astype(f32), (P, 1)),
            "sel": np.tile(sel, (P, 1)),
            "lst": lst, "ust": ust, "slot": slot, "iota": iota,
        })
    return in_maps


def _assemble(results):
    out = np.empty((N, D), np.float32)
    for r in range(N_CORES):
        shard = np.asarray(results[r]["out_shard"]).reshape(
            OROWS, D).astype(np.float32)
        for c, (tk0, ntok, cap, sgo) in enumerate(CHUNK_SPECS):
            nrr = ntok // N_CORES
            t0 = tk0 + r * nrr
            out[t0:t0 + nrr, :] = shard[OUT_OFFS[c]:OUT_OFFS[c] + nrr]
    return out.reshape(B, L, D)


def kernel(x, W1, b1, W2, b2, Wg, bg, k):
    from concourse.bass_utils import run_bass_kernel_spmd

    assert int(k) == 2
    if "nc" not in _cache:
        _cache["nc"] = _build()
    nc = _cache["nc"]
    in_maps = _host_inputs(np.asarray(x), np.asarray(W1), np.asarray(b1),
                           np.asarray(W2), np.asarray(b2), np.asarray(Wg),
                           np.asarray(bg))
    res = run_bass_kernel_spmd(nc, in_maps, core_ids=list(range(N_CORES)),
                               **_cache.get("run_kwargs", {}))
    _cache["last_result"] = res
    return _assemble(res.results)


# revision 22
# speedup vs baseline: 1.0086x; 1.0086x over previous
"""MoE (top-2 of 8 experts) Trainium2 kernel, expert-parallel across 8 cores.

v3 (fully resident weights + pipelined):
  - 4 chunks of 1024 tokens, cap 320, slot groups (0,128,192) -- every W2
    matmul keeps 128 partitions; overlapped slots recompute (free).
  - W1 AND W2 both SBUF-resident (16 MB): the FFN stream never touches
    HBM, so ReduceScatter SDMA traffic cannot starve the Tensor engine.
  - gate: chunk 0 replicated (fp32 PE matmuls); tokens 1024..4095 gated
    data-parallel (each core gates its own 384 tokens; all-expert coefs
    AllGathered as [3072, 8] fp32). AG triggered before chunk-0
    compaction so it completes long before chunk-1 needs it.
  - phase-1 of chunk c+1 (slab, compaction, gather+PE-transpose) emitted
    after FFN(c); the Tile scheduler overlaps it with the FFN stream.
  - per-chunk bf16 ReduceScatter(add); out copies at the end.
"""

import numpy as np
import ml_dtypes

B, L, D, DFF, E = 2, 2048, 1024, 4096, 8
N = B * L                # 4096 tokens
P = 128
KD = D // P              # 8   contraction chunks over D
NDJ = DFF // P           # 32  DFF tiles
# (tk0, ntok, cap, slot-group offsets)
CHUNK_SPECS = [
    (0,    1024, 320, (0, 128, 192)),
    (1024, 1024, 320, (0, 128, 192)),
    (2048, 1024, 320, (0, 128, 192)),
    (3072, 768,  256, (0, 128)),
    (3840, 256,  128, (0,)),
]
NCHUNK = len(CHUNK_SPECS)
N_CORES = 8
OUT_OFFS = [0, 128, 256, 384, 480]   # per-rank output row offsets
OROWS = N // N_CORES                 # 512 output rows per rank
CAPMAX = 320
HALF = D // 2            # 512
DPTOK = 3072 // N_CORES  # 384 data-parallel-gated tokens per core
GCOL = 256               # gate matmul column tile (bf16 rhs)

_cache = {}


def _build():
    import concourse.bass as bass
    import concourse.mybir as mybir
    import concourse.tile as tile
    from concourse import bacc
    from concourse.masks import make_identity

    dt = mybir.dt
    AF = mybir.ActivationFunctionType
    OP = mybir.AluOpType

    nc = bacc.Bacc("TRN2", target_bir_lowering=False, debug=False,
                   num_devices=N_CORES)

    # ---- kernel I/O ----
    x_d = nc.dram_tensor("x", [N, D], dt.bfloat16, kind="ExternalInput")
    # gate inputs: cols 0..1023 = xT of chunk-0 tokens (replicated);
    # cols 1024..1407 = xT of this core's own 384 DP tokens.
    # bf16 hi/lo split planes (exact to ~2^-18 vs fp32).
    xtg_d = nc.dram_tensor("xtg", [P, KD, 2, 1024 + DPTOK], dt.bfloat16,
                           kind="ExternalInput")
    w1_d = nc.dram_tensor("w1", [P, NDJ, KD, P], dt.bfloat16,
                          kind="ExternalInput")
    w2_d = nc.dram_tensor("w2", [P, NDJ, D], dt.bfloat16,
                          kind="ExternalInput")
    b1_d = nc.dram_tensor("b1", [P, NDJ], dt.float32, kind="ExternalInput")
    b2_d = nc.dram_tensor("b2", [P, D], dt.bfloat16, kind="ExternalInput")
    wg_d = nc.dram_tensor("wg", [P, KD, 2, E], dt.bfloat16,
                          kind="ExternalInput")
    bg_d = nc.dram_tensor("bg", [P, E], dt.float32, kind="ExternalInput")
    sel_d = nc.dram_tensor("sel", [P, E], dt.float32, kind="ExternalInput")
    lst_d = nc.dram_tensor("lst", [P, P], dt.float16, kind="ExternalInput")
    ust_d = nc.dram_tensor("ust", [16, 16], dt.float16, kind="ExternalInput")
    slot_d = nc.dram_tensor("slot", [P, CAPMAX], dt.float32,
                            kind="ExternalInput")
    iota_d = nc.dram_tensor("iota", [P, 1], dt.float32, kind="ExternalInput")

    out_d = nc.dram_tensor("out_shard", [OROWS, D], dt.bfloat16,
                           kind="ExternalOutput")

    rg = [list(range(N_CORES))]

    with tile.TileContext(nc) as tc:
        with (
            tc.tile_pool(name="const", bufs=1) as const,
            tc.tile_pool(name="xtgpool", bufs=2) as xtgpool,
            tc.tile_pool(name="xgpool", bufs=2) as xgpool,
            tc.tile_pool(name="xgtpool", bufs=2) as xgtpool,
            tc.tile_pool(name="hpool", bufs=1) as hpool,
            tc.tile_pool(name="ypool", bufs=4) as ypool,
            tc.tile_pool(name="ppool", bufs=2) as ppool,
            tc.tile_pool(name="spool", bufs=3) as spool,
            tc.tile_pool(name="chpool", bufs=2) as chpool,
            tc.tile_pool(name="psum", bufs=1, space="PSUM") as psum,
            tc.tile_pool(name="dram", bufs=1, space="DRAM") as dram,
        ):
            # ---------- constants ----------
            # gate-critical consts load first so gate matmuls start ASAP;
            # the rest stream behind them / on other queues.
            ident = const.tile([P, P], dt.float32, tag="ident")
            make_identity(nc, ident[:])
            identb = const.tile([P, P], dt.bfloat16, tag="identb")
            nc.vector.tensor_copy(identb[:], ident[:])
            identh = const.tile([P, P], dt.float16, tag="identh")
            nc.vector.tensor_copy(identh[:], ident[:])
            wgsb = const.tile([P, KD, 2, E], dt.bfloat16, tag="wgsb")
            nc.sync.dma_start(wgsb[:], wg_d[:])
            bgsb = const.tile([P, E], dt.float32, tag="bgsb")
            nc.sync.dma_start(bgsb[:], bg_d[:])
            selsb = const.tile([P, E], dt.float32, tag="selsb")
            lst = const.tile([P, P], dt.float16, tag="lst")
            ust = const.tile([16, 16], dt.float16, tag="ust")
            slotsb = const.tile([P, CAPMAX], dt.float32, tag="slotsb")
            iotasb = const.tile([P, 1], dt.float32, tag="iotasb")
            b1sb = const.tile([P, NDJ], dt.float32, tag="b1sb")
            b2b = const.tile([P, D], dt.bfloat16, tag="b2b")
            zt = const.tile([P, D], dt.bfloat16, tag="zt")
            nc.vector.memset(zt[:], 0.0)

            # resident weights: W1 on scalar queue, W2 on gpsimd queue
            # (both off the sync queue, which carries the gate loads).
            w1sb = const.tile([P, NDJ, KD, P], dt.bfloat16, tag="w1sb")
            for q in range(8):
                nc.scalar.dma_start(w1sb[:, 4 * q:4 * q + 4, :, :],
                                    w1_d[:, 4 * q:4 * q + 4, :, :])
            w2sb = const.tile([P, NDJ, D], dt.bfloat16, tag="w2sb")
            for q in range(8):
                nc.gpsimd.dma_start(w2sb[:, 4 * q:4 * q + 4, :],
                                    w2_d[:, 4 * q:4 * q + 4, :])

            def late_consts():
                nc.sync.dma_start(selsb[:], sel_d[:])
                nc.sync.dma_start(lst[:], lst_d[:])
                nc.sync.dma_start(ust[:], ust_d[:])
                nc.sync.dma_start(slotsb[:], slot_d[:])
                nc.sync.dma_start(iotasb[:], iota_d[:])
                nc.sync.dma_start(b1sb[:], b1_d[:])
                nc.sync.dma_start(b2b[:], b2_d[:])

            # internal DRAM: per-chunk partial + RS output; gate AG bufs
            partials = []
            rs_outs = []
            for c, (tk0, ntok, cap, sgo) in enumerate(CHUNK_SPECS):
                pc = dram.tile([ntok + 8, D], dt.bfloat16, tag=f"partial{c}")
                partials.append(pc)
                ro = dram.tile([ntok // N_CORES, D], dt.bfloat16,
                               tag=f"rsout{c}")
                rs_outs.append(ro)
            gatebuf = dram.tile([DPTOK // P, P, E], dt.float32,
                                tag="gatebuf")
            gateall = dram.tile([(DPTOK // P) * N_CORES, P, E], dt.float32,
                                tag="gateall")

            # =========== helpers ===========
            def gate(xcols, tpc, nm):
                """logit [P, tpc, E] = (xtg[:,:,xcols:+tpc*128]^T @ Wg) + bg.

                bf16 hi/lo 3-term chains (xh@wh + xl@wh + xh@wl; the
                dropped xl@wl term is ~2^-18), PE-transposed back."""
                logit = chpool.tile([P, tpc, E], dt.float32, tag="logit",
                                    name=f"logit{nm}")
                for col in range(0, tpc * P, GCOL):
                    pw = min(GCOL, tpc * P - col)
                    xtk = xtgpool.tile([P, KD, 2, GCOL], dt.bfloat16,
                                       tag="xtk", name=f"xtk{nm}_{col}")
                    nc.sync.dma_start(
                        xtk[:, :, :, :pw],
                        xtg_d[:, :, :, xcols + col:xcols + col + pw])
                    pgT = psum.tile([E, GCOL], dt.float32, tag="pa", bufs=2,
                                    name=f"pgT{nm}_{col}")
                    nmm = 3 * KD
                    i = 0
                    for kc in range(KD):
                        for (wp, xp) in ((0, 0), (1, 0), (0, 1)):
                            nc.tensor.matmul(pgT[:, :pw],
                                             lhsT=wgsb[:, kc, wp, :],
                                             rhs=xtk[:, kc, xp, :pw],
                                             start=(i == 0),
                                             stop=(i == nmm - 1))
                            i += 1
                    lgs = spool.tile([E, GCOL], dt.float32, tag="lgs",
                                     bufs=1)
                    nc.vector.tensor_copy(lgs[:, :pw], pgT[:, :pw])
                    for fo in range(pw // P):
                        f = (col + fo * P) // P
                        ptb = psum.tile([P, E], dt.float32, tag="pa",
                                        bufs=2, name=f"ptb{nm}_{f}")
                        nc.tensor.matmul(ptb[:],
                                         lhsT=lgs[:, fo * P:(fo + 1) * P],
                                         rhs=ident[:E, :E],
                                         is_transpose=True,
                                         start=True, stop=True)
                        nc.vector.tensor_add(logit[:, f, :], ptb[:], bgsb[:])
                return logit

            def softmax_own(logit, tpc, nm):
                """baseline top-2-of-8 for OUR expert: mask f16 + coef f32."""
                mask_ch = chpool.tile([P, tpc], dt.float16, tag="mask",
                                      name=f"mask{nm}")
                coef_ch = chpool.tile([P, tpc], dt.float32, tag="coef",
                                      name=f"coef{nm}")
                m1 = spool.tile([P, tpc], dt.float32, tag="m1")
                nc.vector.reduce_max(m1[:], logit[:],
                                     axis=mybir.AxisListType.X)
                eqm = spool.tile([P, tpc, E], dt.float32, tag="eqm")
                nc.vector.tensor_tensor(
                    eqm[:], logit[:],
                    m1[:, :, None].to_broadcast([P, tpc, E]), op=OP.is_ge)
                nc.vector.tensor_scalar_mul(eqm[:], eqm[:], 1e9)
                nc.vector.tensor_sub(eqm[:], logit[:], eqm[:])
                m2 = spool.tile([P, tpc], dt.float32, tag="m2")
                nc.vector.reduce_max(m2[:], eqm[:], axis=mybir.AxisListType.X)
                exps = spool.tile([P, tpc, E], dt.float32, tag="exps")
                nc.scalar.activation(exps[:], logit[:], AF.Exp)
                ssum = spool.tile([P, tpc], dt.float32, tag="ssum")
                nc.vector.reduce_sum(ssum[:], exps[:],
                                     axis=mybir.AxisListType.X)
                rinv = spool.tile([P, tpc], dt.float32, tag="rinv")
                nc.vector.reciprocal(rinv[:], ssum[:])
                selb = selsb[:, None, :].to_broadcast([P, tpc, E])
                tmp = spool.tile([P, tpc, E], dt.float32, tag="tmp")
                nc.vector.tensor_mul(tmp[:], logit[:], selb)
                lour = spool.tile([P, tpc], dt.float32, tag="lour")
                nc.vector.reduce_sum(lour[:], tmp[:],
                                     axis=mybir.AxisListType.X)
                nc.vector.tensor_mul(tmp[:], exps[:], selb)
                eour = spool.tile([P, tpc], dt.float32, tag="eour")
                nc.vector.reduce_sum(eour[:], tmp[:],
                                     axis=mybir.AxisListType.X)
                nc.vector.tensor_tensor(mask_ch[:], lour[:], m2[:],
                                        op=OP.is_ge)
                nc.vector.tensor_mul(coef_ch[:], eour[:], rinv[:])
                nc.vector.tensor_mul(coef_ch[:], coef_ch[:], mask_ch[:])
                return mask_ch, coef_ch

            def softmax_all(logit, tpc, nm):
                """all-expert top-2 coefs [P, tpc, E] (for the DP slice)."""
                coefE = spool.tile([P, tpc, E], dt.float32, tag="coefE",
                                   name=f"coefE{nm}", bufs=1)
                m1 = spool.tile([P, tpc], dt.float32, tag="m1")
                nc.vector.reduce_max(m1[:], logit[:],
                                     axis=mybir.AxisListType.X)
                eqm = spool.tile([P, tpc, E], dt.float32, tag="eqm")
                nc.vector.tensor_tensor(
                    eqm[:], logit[:],
                    m1[:, :, None].to_broadcast([P, tpc, E]), op=OP.is_ge)
                nc.vector.tensor_scalar_mul(eqm[:], eqm[:], 1e9)
                nc.vector.tensor_sub(eqm[:], logit[:], eqm[:])
                m2 = spool.tile([P, tpc], dt.float32, tag="m2")
                nc.vector.reduce_max(m2[:], eqm[:], axis=mybir.AxisListType.X)
                exps = spool.tile([P, tpc, E], dt.float32, tag="exps")
                nc.scalar.activation(exps[:], logit[:], AF.Exp)
                ssum = spool.tile([P, tpc], dt.float32, tag="ssum")
                nc.vector.reduce_sum(ssum[:], exps[:],
                                     axis=mybir.AxisListType.X)
                rinv = spool.tile([P, tpc], dt.float32, tag="rinv")
                nc.vector.reciprocal(rinv[:], ssum[:])
                nc.vector.tensor_tensor(
                    coefE[:], logit[:],
                    m2[:, :, None].to_broadcast([P, tpc, E]), op=OP.is_ge)
                nc.vector.tensor_mul(coefE[:], coefE[:], exps[:])
                nc.vector.tensor_mul(
                    coefE[:], coefE[:],
                    rinv[:, :, None].to_broadcast([P, tpc, E]))
                return coefE

            def compact(c, mask_t, coef_t):
                """stream-compact selected tokens; returns idx/coef tiles."""
                tk0, ntok, cap, sgo = CHUNK_SPECS[c]
                tpc = ntok // P
                nsg = len(sgo)
                mask_ch = mask_t[:, :tpc]
                coef_ch = coef_t[:, :tpc]
                mt_ps = psum.tile([16, P], dt.float16, tag="pc", bufs=1,
                                  name=f"mtps{c}")
                nc.tensor.matmul(mt_ps[:tpc, :], lhsT=mask_ch,
                                 rhs=identh[:], is_transpose=True,
                                 start=True, stop=True)
                mts = spool.tile([16, P], dt.float16, tag="mts")
                nc.vector.tensor_copy(mts[:tpc, :], mt_ps[:tpc, :])
                cs = spool.tile([16, 1], dt.float32, tag="cs")
                nc.vector.reduce_sum(cs[:tpc], mts[:tpc, :],
                                     axis=mybir.AxisListType.X)
                cs_b = spool.tile([16, P], dt.float16, tag="cs_b")
                nc.vector.tensor_copy(cs_b[:tpc],
                                      cs[:tpc].to_broadcast([tpc, P]))
                ppos = psum.tile([P, 16], dt.float32, tag="pc", bufs=1,
                                 name=f"ppos{c}")
                nc.tensor.matmul(ppos[:, :tpc], lhsT=lst[:], rhs=mask_ch,
                                 start=True, stop=False)
                nc.tensor.matmul(ppos[:, :tpc], lhsT=cs_b[:tpc],
                                 rhs=ust[:tpc, :tpc],
                                 start=False, stop=True)
                t1 = spool.tile([P, 16], dt.float32, tag="t1")
                nc.vector.tensor_scalar_add(t1[:, :tpc], ppos[:, :tpc],
                                            -float(cap))
                t2 = spool.tile([P, 16], dt.float32, tag="t2")
                nc.vector.tensor_mul(t2[:, :tpc], t1[:, :tpc], mask_ch)
                pos_eff = chpool.tile([P, 16], dt.float32, tag="pos_eff",
                                      name=f"pos_eff{c}")
                nc.vector.tensor_scalar_add(pos_eff[:, :tpc], t2[:, :tpc],
                                            float(cap))

                pcmp = psum.tile([P, 3 * 3], dt.float32, tag="pc",
                                 bufs=1, name=f"pcmp{c}")
                for f in range(tpc):
                    perm = ppool.tile([P, CAPMAX], dt.float16, tag="perm",
                                      name=f"perm{c}_{f}")
                    nc.vector.tensor_tensor(
                        perm[:, :cap],
                        pos_eff[:, f:f + 1].to_broadcast([P, cap]),
                        slotsb[:, :cap], op=OP.is_equal)
                    rhs3 = spool.tile([P, 3], dt.float16, tag="rhs3")
                    nc.vector.tensor_scalar_add(rhs3[:, 0:1], iotasb[:],
                                                float(f * P))
                    nc.vector.tensor_copy(rhs3[:, 1:2], coef_ch[:, f:f + 1])
                    nc.vector.memset(rhs3[:, 2:3], 1.0)
                    for sg in range(nsg):
                        nc.tensor.matmul(
                            pcmp[:, 3 * sg:3 * sg + 3],
                            lhsT=perm[:, sgo[sg]:sgo[sg] + P],
                            rhs=rhs3[:],
                            start=(f == 0 and sg == 0),
                            stop=(f == tpc - 1 and sg == nsg - 1))

                idx_g_i = chpool.tile([P, 3], dt.int32, tag="idx_g",
                                      name=f"idxg{c}")
                idx_s_i = chpool.tile([P, 3], dt.int32, tag="idx_s",
                                      name=f"idxs{c}")
                coef_sg = chpool.tile([P, 3], dt.float32, tag="coef_sg",
                                      name=f"coefsg{c}")
                for sg in range(nsg):
                    cmp = spool.tile([P, 3], dt.float32, tag="cmp")
                    nc.vector.tensor_copy(cmp[:], pcmp[:, 3 * sg:3 * sg + 3])
                    nc.vector.tensor_copy(coef_sg[:, sg:sg + 1], cmp[:, 1:2])
                    gidx = spool.tile([P, 1], dt.float32, tag="gidx")
                    nc.vector.tensor_scalar_add(gidx[:], cmp[:, 0:1],
                                                float(tk0))
                    nc.vector.tensor_copy(idx_g_i[:, sg:sg + 1], gidx[:])
                    iv = spool.tile([P, 1], dt.float32, tag="iv")
                    nc.vector.tensor_scalar(iv[:], cmp[:, 2:3],
                                            -float(ntok), float(ntok),
                                            op0=OP.mult, op1=OP.add)
                    sidx = spool.tile([P, 1], dt.float32, tag="sidx")
                    nc.vector.tensor_add(sidx[:], cmp[:, 0:1], iv[:])
                    nc.vector.tensor_copy(idx_s_i[:, sg:sg + 1], sidx[:])
                return idx_g_i, idx_s_i, coef_sg

            def gather(c, idx_g_i):
                """indirect-gather selected bf16 rows + PE transpose."""
                tk0, ntok, cap, sgo = CHUNK_SPECS[c]
                nsg = len(sgo)
                xgT = xgtpool.tile([P, KD, CAPMAX], dt.bfloat16, tag="xgT",
                                   name=f"xgT{c}")
                xgs = []
                for sg in range(nsg):
                    xg = xgpool.tile([P, D], dt.bfloat16, tag="xg", bufs=3,
                                     name=f"xg{c}_{sg}")
                    nc.gpsimd.indirect_dma_start(
                        out=xg[:], out_offset=None, in_=x_d[:, :],
                        in_offset=bass.IndirectOffsetOnAxis(
                            ap=idx_g_i[:, sg:sg + 1], axis=0))
                    xgs.append(xg)
                for sg in range(nsg):
                    for g in range(KD // 2):
                        pt2 = psum.tile([P, 2, P], dt.bfloat16, tag="pa",
                                        bufs=2, name=f"pt2_{c}_{sg}_{g}")
                        for j in range(2):
                            kc = 2 * g + j
                            nc.tensor.matmul(
                                pt2[:, j, :],
                                lhsT=xgs[sg][:, kc * P:(kc + 1) * P],
                                rhs=identb[:], is_transpose=True,
                                start=(j == 0), stop=(j == 1))
                        nc.vector.tensor_copy(
                            xgT[:, 2 * g:2 * g + 2, sgo[sg]:sgo[sg] + P],
                            pt2[:])
                return xgT

            def zero_partial(c):
                tk0, ntok, cap, sgo = CHUNK_SPECS[c]
                for i in range(ntok // P):
                    nc.sync.dma_start(
                        partials[c][i * P:(i + 1) * P, :], zt[:])

            def ffn_w1(c, xgT):
                tk0, ntok, cap, sgo = CHUNK_SPECS[c]
                hT = hpool.tile([P, NDJ, CAPMAX], dt.bfloat16, tag="hT",
                                name=f"hT{c}")
                for dj in range(NDJ):
                    ph = psum.tile([P, 320], dt.float32, tag="ph", bufs=2,
                                   name=f"ph{c}_{dj}")
                    for kc in range(KD):
                        nc.tensor.matmul(
                            ph[:, :cap],
                            lhsT=w1sb[:, dj, kc, :],
                            rhs=xgT[:, kc, :cap],
                            start=(kc == 0), stop=(kc == KD - 1))
                    nc.scalar.activation(hT[:, dj, :cap], ph[:, :cap],
                                         AF.Relu, bias=b1sb[:, dj:dj + 1])
                return hT

            def ffn_w2(c, hT, idx_s_i, coef_sg):
                tk0, ntok, cap, sgo = CHUNK_SPECS[c]
                nsg = len(sgo)
                youts = [ypool.tile([P, D], dt.bfloat16, tag="yout",
                                    name=f"yout{c}_{i}") for i in range(nsg)]
                for sg in range(nsg):
                    for h in range(2):
                        hs = slice(h * HALF, (h + 1) * HALF)
                        py = psum.tile([P, HALF], dt.float32, tag="pys",
                                       bufs=3, name=f"py{c}_{sg}_{h}")
                        for dj in range(NDJ):
                            nc.tensor.matmul(
                                py[:],
                                lhsT=hT[:, dj, sgo[sg]:sgo[sg] + P],
                                rhs=w2sb[:, dj, hs],
                                start=(dj == 0), stop=(dj == NDJ - 1))
                        ytmp = spool.tile([P, HALF], dt.float32, tag="ytmp",
                                          bufs=1)
                        nc.vector.tensor_add(ytmp[:], py[:], b2b[:, hs])
                        nc.vector.tensor_scalar_mul(youts[sg][:, hs],
                                                    ytmp[:],
                                                    coef_sg[:, sg:sg + 1])
                for sg in range(nsg):
                    nc.gpsimd.indirect_dma_start(
                        out=partials[c][:, :],
                        out_offset=bass.IndirectOffsetOnAxis(
                            ap=idx_s_i[:, sg:sg + 1], axis=0),
                        in_=youts[sg][:], in_offset=None)
                nc.gpsimd.collective_compute(
                    "ReduceScatter", mybir.AluOpType.add, replica_groups=rg,
                    ins=[partials[c][0:ntok, :].opt()],
                    outs=[rs_outs[c][:, :].opt()])

            def phase1_dp(c):
                """slab load + mask/coef + compact + gather for chunk c."""
                tk0, ntok, cap, sgo = CHUNK_SPECS[c]
                tpc = ntok // P
                slab = spool.tile([P, 8, E], dt.float32, tag="slab",
                                  name=f"slab{c}", bufs=2)
                f0 = (tk0 - 1024) // P
                # gpsimd queue: ordered behind the AllGather trigger, so
                # the collective-completion wait is enforced by queue order
                # (cross-queue sem waits on collectives proved racy).
                for f in range(tpc):
                    nc.gpsimd.dma_start(slab[:, f, :], gateall[f0 + f, :, :])
                selb = selsb[:, None, :].to_broadcast([P, tpc, E])
                tmp = spool.tile([P, 8, E], dt.float32, tag="tmpsl")
                nc.vector.tensor_mul(tmp[:, :tpc, :], slab[:, :tpc, :], selb)
                coef_ch = chpool.tile([P, 8], dt.float32, tag="coef",
                                      name=f"coef{c}")
                nc.vector.reduce_sum(coef_ch[:, :tpc], tmp[:, :tpc, :],
                                     axis=mybir.AxisListType.X)
                mask_ch = chpool.tile([P, 8], dt.float16, tag="mask",
                                      name=f"mask{c}")
                nc.vector.tensor_scalar(mask_ch[:, :tpc], coef_ch[:, :tpc],
                                        0.0, None, op0=OP.is_gt)
                idx_g_i, idx_s_i, coef_sg = compact(c, mask_ch, coef_ch)
                zero_partial(c)
                xgT = gather(c, idx_g_i)
                return xgT, idx_s_i, coef_sg

            # ============ emission ============
            # --- DP gate slice first (AG triggers early, completes in the
            # shadow of chunk-0's replicated gate + FFN) ---
            logitD = gate(1024, DPTOK // P, "dp")
            coefE = softmax_all(logitD, DPTOK // P, "dp")
            for f in range(DPTOK // P):
                nc.scalar.dma_start(gatebuf[f, :, :], coefE[:, f, :])
            nc.gpsimd.collective_compute(
                "AllGather", mybir.AluOpType.bypass, replica_groups=rg,
                ins=[gatebuf[:, :, :].opt()],
                outs=[gateall[:, :, :].opt()])

            # --- chunk 0: replicated gate + compact + gather ---
            logit0 = gate(0, 8, "c0")
            late_consts()
            mask0, coef0 = softmax_own(logit0, 8, "c0")
            idx_g0, idx_s0, coef_sg0 = compact(0, mask0, coef0)
            zero_partial(0)
            xgT0 = gather(0, idx_g0)

            # --- pipelined FFN ---
            prev = (xgT0, idx_s0, coef_sg0)
            for c in range(NCHUNK):
                xgT, idx_s_i, coef_sg = prev
                hT = ffn_w1(c, xgT)
                ffn_w2(c, hT, idx_s_i, coef_sg)
                if c + 1 < NCHUNK:
                    prev = phase1_dp(c + 1)

            # out copies (earlier RS long done; last copy is the tail)
            for c, (tk0, ntok, cap, sgo) in enumerate(CHUNK_SPECS):
                nrr = ntok // N_CORES
                nc.gpsimd.dma_start(
                    out_d[OUT_OFFS[c]:OUT_OFFS[c] + nrr, :], rs_outs[c][:, :])

    nc.compile()
    return nc


def _host_inputs(x, W1, b1, W2, b2, Wg, bg):
    bf16 = ml_dtypes.bfloat16
    f32 = np.float32
    f16 = np.float16
    x2 = np.ascontiguousarray(x.reshape(N, D), dtype=f32)
    xb = x2.astype(bf16)
    xt = np.ascontiguousarray(x2.T)           # [D, N]
    xtt = xt.reshape(KD, P, N)                # [kc, p, tok]
    lst = np.triu(np.ones((P, P), f16), k=1)
    ust = np.triu(np.ones((16, 16), f16), k=1)
    slot = np.tile(np.arange(CAPMAX, dtype=f32), (P, 1))
    iota = np.arange(P, dtype=f32).reshape(P, 1)
    wgt = np.ascontiguousarray(
        Wg.reshape(KD, P, E).transpose(1, 0, 2))          # [P, KD, E] f32
    wg_hi = wgt.astype(bf16)
    wg_lo = (wgt - wg_hi.astype(f32)).astype(bf16)
    wg = np.stack([wg_hi, wg_lo], axis=2)                 # [P, KD, 2, E]
    in_maps = []
    for e in range(N_CORES):
        sel = np.zeros((E,), f32)
        sel[e] = 1.0
        xtgf = np.empty((P, KD, 1024 + DPTOK), f32)
        xtgf[:, :, :1024] = xtt[:, :, :1024].transpose(1, 0, 2)
        t0 = 1024 + e * DPTOK
        xtgf[:, :, 1024:] = xtt[:, :, t0:t0 + DPTOK].transpose(1, 0, 2)
        xtg_hi = xtgf.astype(bf16)
        xtg_lo = (xtgf - xtg_hi.astype(f32)).astype(bf16)
        xtg = np.stack([xtg_hi, xtg_lo], axis=2)    # [P, KD, 2, cols]
        in_maps.append({
            "x": xb,
            "xtg": xtg,
            "w1": np.ascontiguousarray(
                W1[e].reshape(KD, P, NDJ, P).transpose(1, 2, 0, 3)
            ).astype(bf16),
            "w2": np.ascontiguousarray(
                W2[e].reshape(NDJ, P, D).transpose(1, 0, 2)).astype(bf16),
            "b1": np.ascontiguousarray(
                b1[e].reshape(NDJ, P).T).astype(f32),
            "b2": np.tile(b2[e].astype(bf16), (P, 1)),
            "wg": wg,
            "bg": np.tile(bg.astype(f32), (P, 1)),
            "sel": np.tile(sel, (P, 1)),
            "lst": lst, "ust": ust, "slot": slot, "iota": iota,
        })
    return in_maps


def _assemble(results):
    out = np.empty((N, D), np.float32)
    for r in range(N_CORES):
        shard = np.asarray(results[r]["out_shard"]).reshape(
            OROWS, D).astype(np.float32)
        for c, (tk0, ntok, cap, sgo) in enumerate(CHUNK_SPECS):
            nrr = ntok // N_CORES
            t0 = tk0 + r * nrr
            out[t0:t0 + nrr, :] = shard[OUT_OFFS[c]:OUT_OFFS[c] + nrr]
    return out.reshape(B, L, D)


def kernel(x, W1, b1, W2, b2, Wg, bg, k):
    from concourse.bass_utils import run_bass_kernel_spmd

    assert int(k) == 2
    if "nc" not in _cache:
        _cache["nc"] = _build()
    nc = _cache["nc"]
    in_maps = _host_inputs(np.asarray(x), np.asarray(W1), np.asarray(b1),
                           np.asarray(W2), np.asarray(b2), np.asarray(Wg),
                           np.asarray(bg))
    res = run_bass_kernel_spmd(nc, in_maps, core_ids=list(range(N_CORES)),
                               **_cache.get("run_kwargs", {}))
    _cache["last_result"] = res
    return _assemble(res.results)


# revision 23
# speedup vs baseline: 1.0850x; 1.0758x over previous
"""MoE (top-2 of 8 experts) Trainium2 kernel, expert-parallel across 8 cores.

v3 (fully resident weights + pipelined):
  - 4 chunks of 1024 tokens, cap 320, slot groups (0,128,192) -- every W2
    matmul keeps 128 partitions; overlapped slots recompute (free).
  - W1 AND W2 both SBUF-resident (16 MB): the FFN stream never touches
    HBM, so ReduceScatter SDMA traffic cannot starve the Tensor engine.
  - gate: chunk 0 replicated (fp32 PE matmuls); tokens 1024..4095 gated
    data-parallel (each core gates its own 384 tokens; all-expert coefs
    AllGathered as [3072, 8] fp32). AG triggered before chunk-0
    compaction so it completes long before chunk-1 needs it.
  - phase-1 of chunk c+1 (slab, compaction, gather+PE-transpose) emitted
    after FFN(c); the Tile scheduler overlaps it with the FFN stream.
  - per-chunk bf16 ReduceScatter(add); out copies at the end.
"""

import numpy as np
import ml_dtypes

B, L, D, DFF, E = 2, 2048, 1024, 4096, 8
N = B * L                # 4096 tokens
P = 128
KD = D // P              # 8   contraction chunks over D
NDJ = DFF // P           # 32  DFF tiles
# (tk0, ntok, cap, slot-group offsets)
CHUNK_SPECS = [
    (0,    1024, 320, (0, 128, 192)),
    (1024, 1024, 320, (0, 128, 192)),
    (2048, 1024, 320, (0, 128, 192)),
    (3072, 768,  256, (0, 128)),
    (3840, 256,  128, (0,)),
]
NCHUNK = len(CHUNK_SPECS)
N_CORES = 8
OUT_OFFS = [0, 128, 256, 384, 480]   # per-rank output row offsets
OROWS = N // N_CORES                 # 512 output rows per rank
CAPMAX = 320
HALF = D // 2            # 512
DPTOK = 3072 // N_CORES  # 384 data-parallel-gated tokens per core
GCOL = 256               # gate matmul column tile (bf16 rhs)

_cache = {}


def _build():
    import concourse.bass as bass
    import concourse.mybir as mybir
    import concourse.tile as tile
    from concourse import bacc
    from concourse.masks import make_identity

    dt = mybir.dt
    AF = mybir.ActivationFunctionType
    OP = mybir.AluOpType

    nc = bacc.Bacc("TRN2", target_bir_lowering=False, debug=False,
                   num_devices=N_CORES)

    # ---- kernel I/O ----
    x_d = nc.dram_tensor("x", [N, D], dt.bfloat16, kind="ExternalInput")
    # gate inputs: cols 0..1023 = xT of chunk-0 tokens (replicated);
    # cols 1024..1407 = xT of this core's own 384 DP tokens.
    # bf16 hi/lo split planes (exact to ~2^-18 vs fp32).
    xtg_d = nc.dram_tensor("xtg", [P, KD, 2, 1024 + DPTOK], dt.bfloat16,
                           kind="ExternalInput")
    w1_d = nc.dram_tensor("w1", [P, NDJ, KD, P], dt.bfloat16,
                          kind="ExternalInput")
    w2_d = nc.dram_tensor("w2", [P, NDJ, D], dt.bfloat16,
                          kind="ExternalInput")
    b1_d = nc.dram_tensor("b1", [P, NDJ], dt.float32, kind="ExternalInput")
    b2_d = nc.dram_tensor("b2", [P, D], dt.bfloat16, kind="ExternalInput")
    wg_d = nc.dram_tensor("wg", [P, KD, 2, E], dt.bfloat16,
                          kind="ExternalInput")
    bg_d = nc.dram_tensor("bg", [P, E], dt.float32, kind="ExternalInput")
    sel_d = nc.dram_tensor("sel", [P, E], dt.float32, kind="ExternalInput")
    lst_d = nc.dram_tensor("lst", [P, P], dt.float16, kind="ExternalInput")
    ust_d = nc.dram_tensor("ust", [16, 16], dt.float16, kind="ExternalInput")
    slot_d = nc.dram_tensor("slot", [P, CAPMAX], dt.float32,
                            kind="ExternalInput")
    iota_d = nc.dram_tensor("iota", [P, 1], dt.float32, kind="ExternalInput")

    out_d = nc.dram_tensor("out_shard", [OROWS, D], dt.bfloat16,
                           kind="ExternalOutput")

    rg = [list(range(N_CORES))]

    with tile.TileContext(nc) as tc:
        with (
            tc.tile_pool(name="const", bufs=1) as const,
            tc.tile_pool(name="xtgpool", bufs=2) as xtgpool,
            tc.tile_pool(name="xgpool", bufs=2) as xgpool,
            tc.tile_pool(name="xgtpool", bufs=2) as xgtpool,
            tc.tile_pool(name="hpool", bufs=1) as hpool,
            tc.tile_pool(name="ypool", bufs=4) as ypool,
            tc.tile_pool(name="ppool", bufs=2) as ppool,
            tc.tile_pool(name="spool", bufs=3) as spool,
            tc.tile_pool(name="chpool", bufs=2) as chpool,
            tc.tile_pool(name="psum", bufs=1, space="PSUM") as psum,
            tc.tile_pool(name="dram", bufs=1, space="DRAM") as dram,
        ):
            # ---------- constants ----------
            # gate-critical consts load first so gate matmuls start ASAP;
            # the rest stream behind them / on other queues.
            ident = const.tile([P, P], dt.float32, tag="ident")
            make_identity(nc, ident[:])
            identb = const.tile([P, P], dt.bfloat16, tag="identb")
            nc.vector.tensor_copy(identb[:], ident[:])
            identh = const.tile([P, P], dt.float16, tag="identh")
            nc.vector.tensor_copy(identh[:], ident[:])
            wgsb = const.tile([P, KD, 2, E], dt.bfloat16, tag="wgsb")
            nc.sync.dma_start(wgsb[:], wg_d[:])
            bgsb = const.tile([P, E], dt.float32, tag="bgsb")
            nc.sync.dma_start(bgsb[:], bg_d[:])
            selsb = const.tile([P, E], dt.float32, tag="selsb")
            lst = const.tile([P, P], dt.float16, tag="lst")
            ust = const.tile([16, 16], dt.float16, tag="ust")
            slotsb = const.tile([P, CAPMAX], dt.float32, tag="slotsb")
            iotasb = const.tile([P, 1], dt.float32, tag="iotasb")
            b1sb = const.tile([P, NDJ], dt.float32, tag="b1sb")
            b2b = const.tile([P, D], dt.bfloat16, tag="b2b")
            zt = const.tile([P, D], dt.bfloat16, tag="zt")
            nc.vector.memset(zt[:], 0.0)

            # resident weights: W1 on scalar queue, W2 on gpsimd queue
            # (both off the sync queue, which carries the gate loads).
            w1sb = const.tile([P, NDJ, KD, P], dt.bfloat16, tag="w1sb")
            for q in range(8):
                nc.scalar.dma_start(w1sb[:, 4 * q:4 * q + 4, :, :],
                                    w1_d[:, 4 * q:4 * q + 4, :, :])
            w2sb = const.tile([P, NDJ, D], dt.bfloat16, tag="w2sb")
            for q in range(8):
                nc.gpsimd.dma_start(w2sb[:, 4 * q:4 * q + 4, :],
                                    w2_d[:, 4 * q:4 * q + 4, :])

            def late_consts():
                nc.sync.dma_start(selsb[:], sel_d[:])
                nc.sync.dma_start(lst[:], lst_d[:])
                nc.sync.dma_start(ust[:], ust_d[:])
                nc.sync.dma_start(slotsb[:], slot_d[:])
                nc.sync.dma_start(iotasb[:], iota_d[:])
                nc.sync.dma_start(b1sb[:], b1_d[:])
                nc.sync.dma_start(b2b[:], b2_d[:])

            # internal DRAM: per-chunk partial + RS output; gate AG bufs
            partials = []
            rs_outs = []
            for c, (tk0, ntok, cap, sgo) in enumerate(CHUNK_SPECS):
                pc = dram.tile([ntok + 8, D], dt.bfloat16, tag=f"partial{c}")
                partials.append(pc)
                ro = dram.tile([ntok // N_CORES, D], dt.bfloat16,
                               tag=f"rsout{c}")
                rs_outs.append(ro)
            gatebuf = dram.tile([DPTOK // P, P, E], dt.float32,
                                tag="gatebuf")
            gateall = dram.tile([(DPTOK // P) * N_CORES, P, E], dt.float32,
                                tag="gateall")

            # =========== helpers ===========
            def gate(xcols, tpc, nm):
                """logit [P, tpc, E] = (xtg[:,:,xcols:+tpc*128]^T @ Wg) + bg.

                bf16 hi/lo 3-term chains (xh@wh + xl@wh + xh@wl; the
                dropped xl@wl term is ~2^-18), PE-transposed back."""
                logit = chpool.tile([P, tpc, E], dt.float32, tag="logit",
                                    name=f"logit{nm}")
                for col in range(0, tpc * P, GCOL):
                    pw = min(GCOL, tpc * P - col)
                    xtk = xtgpool.tile([P, KD, 2, GCOL], dt.bfloat16,
                                       tag="xtk", name=f"xtk{nm}_{col}")
                    nc.sync.dma_start(
                        xtk[:, :, :, :pw],
                        xtg_d[:, :, :, xcols + col:xcols + col + pw])
                    pgT = psum.tile([E, GCOL], dt.float32, tag="pa", bufs=2,
                                    name=f"pgT{nm}_{col}")
                    nmm = 3 * KD
                    i = 0
                    for kc in range(KD):
                        for (wp, xp) in ((0, 0), (1, 0), (0, 1)):
                            nc.tensor.matmul(pgT[:, :pw],
                                             lhsT=wgsb[:, kc, wp, :],
                                             rhs=xtk[:, kc, xp, :pw],
                                             start=(i == 0),
                                             stop=(i == nmm - 1))
                            i += 1
                    lgs = spool.tile([E, GCOL], dt.float32, tag="lgs",
                                     bufs=1)
                    nc.vector.tensor_copy(lgs[:, :pw], pgT[:, :pw])
                    for fo in range(pw // P):
                        f = (col + fo * P) // P
                        ptb = psum.tile([P, E], dt.float32, tag="pa",
                                        bufs=2, name=f"ptb{nm}_{f}")
                        nc.tensor.matmul(ptb[:],
                                         lhsT=lgs[:, fo * P:(fo + 1) * P],
                                         rhs=ident[:E, :E],
                                         is_transpose=True,
                                         start=True, stop=True)
                        nc.vector.tensor_add(logit[:, f, :], ptb[:], bgsb[:])
                return logit

            def softmax_own(logit, tpc, nm):
                """baseline top-2-of-8 for OUR expert: mask f16 + coef f32."""
                mask_ch = chpool.tile([P, tpc], dt.float16, tag="mask",
                                      name=f"mask{nm}")
                coef_ch = chpool.tile([P, tpc], dt.float32, tag="coef",
                                      name=f"coef{nm}")
                m1 = spool.tile([P, tpc], dt.float32, tag="m1")
                nc.vector.reduce_max(m1[:], logit[:],
                                     axis=mybir.AxisListType.X)
                eqm = spool.tile([P, tpc, E], dt.float32, tag="eqm")
                nc.vector.tensor_tensor(
                    eqm[:], logit[:],
                    m1[:, :, None].to_broadcast([P, tpc, E]), op=OP.is_ge)
                nc.vector.tensor_scalar_mul(eqm[:], eqm[:], 1e9)
                nc.vector.tensor_sub(eqm[:], logit[:], eqm[:])
                m2 = spool.tile([P, tpc], dt.float32, tag="m2")
                nc.vector.reduce_max(m2[:], eqm[:], axis=mybir.AxisListType.X)
                exps = spool.tile([P, tpc, E], dt.float32, tag="exps")
                nc.scalar.activation(exps[:], logit[:], AF.Exp)
                ssum = spool.tile([P, tpc], dt.float32, tag="ssum")
                nc.vector.reduce_sum(ssum[:], exps[:],
                                     axis=mybir.AxisListType.X)
                rinv = spool.tile([P, tpc], dt.float32, tag="rinv")
                nc.vector.reciprocal(rinv[:], ssum[:])
                selb = selsb[:, None, :].to_broadcast([P, tpc, E])
                tmp = spool.tile([P, tpc, E], dt.float32, tag="tmp")
                nc.vector.tensor_mul(tmp[:], logit[:], selb)
                lour = spool.tile([P, tpc], dt.float32, tag="lour")
                nc.vector.reduce_sum(lour[:], tmp[:],
                                     axis=mybir.AxisListType.X)
                nc.vector.tensor_mul(tmp[:], exps[:], selb)
                eour = spool.tile([P, tpc], dt.float32, tag="eour")
                nc.vector.reduce_sum(eour[:], tmp[:],
                                     axis=mybir.AxisListType.X)
                nc.vector.tensor_tensor(mask_ch[:], lour[:], m2[:],
                                        op=OP.is_ge)
                nc.vector.tensor_mul(coef_ch[:], eour[:], rinv[:])
                nc.vector.tensor_mul(coef_ch[:], coef_ch[:], mask_ch[:])
                return mask_ch, coef_ch

            def softmax_all(logit, tpc, nm):
                """all-expert top-2 coefs [P, tpc, E] (for the DP slice)."""
                coefE = spool.tile([P, tpc, E], dt.float32, tag="coefE",
                                   name=f"coefE{nm}", bufs=1)
                m1 = spool.tile([P, tpc], dt.float32, tag="m1")
                nc.vector.reduce_max(m1[:], logit[:],
                                     axis=mybir.AxisListType.X)
                eqm = spool.tile([P, tpc, E], dt.float32, tag="eqm")
                nc.vector.tensor_tensor(
                    eqm[:], logit[:],
                    m1[:, :, None].to_broadcast([P, tpc, E]), op=OP.is_ge)
                nc.vector.tensor_scalar_mul(eqm[:], eqm[:], 1e9)
                nc.vector.tensor_sub(eqm[:], logit[:], eqm[:])
                m2 = spool.tile([P, tpc], dt.float32, tag="m2")
                nc.vector.reduce_max(m2[:], eqm[:], axis=mybir.AxisListType.X)
                exps = spool.tile([P, tpc, E], dt.float32, tag="exps")
                nc.scalar.activation(exps[:], logit[:], AF.Exp)
                ssum = spool.tile([P, tpc], dt.float32, tag="ssum")
                nc.vector.reduce_sum(ssum[:], exps[:],
                                     axis=mybir.AxisListType.X)
                rinv = spool.tile([P, tpc], dt.float32, tag="rinv")
                nc.vector.reciprocal(rinv[:], ssum[:])
                nc.vector.tensor_tensor(
                    coefE[:], logit[:],
                    m2[:, :, None].to_broadcast([P, tpc, E]), op=OP.is_ge)
                nc.vector.tensor_mul(coefE[:], coefE[:], exps[:])
                nc.vector.tensor_mul(
                    coefE[:], coefE[:],
                    rinv[:, :, None].to_broadcast([P, tpc, E]))
                return coefE

            def compact(c, mask_t, coef_t):
                """stream-compact selected tokens; returns idx/coef tiles."""
                tk0, ntok, cap, sgo = CHUNK_SPECS[c]
                tpc = ntok // P
                nsg = len(sgo)
                mask_ch = mask_t[:, :tpc]
                coef_ch = coef_t[:, :tpc]
                mt_ps = psum.tile([16, P], dt.float16, tag="pc", bufs=1,
                                  name=f"mtps{c}")
                nc.tensor.matmul(mt_ps[:tpc, :], lhsT=mask_ch,
                                 rhs=identh[:], is_transpose=True,
                                 start=True, stop=True)
                mts = spool.tile([16, P], dt.float16, tag="mts")
                nc.vector.tensor_copy(mts[:tpc, :], mt_ps[:tpc, :])
                cs = spool.tile([16, 1], dt.float32, tag="cs")
                nc.vector.reduce_sum(cs[:tpc], mts[:tpc, :],
                                     axis=mybir.AxisListType.X)
                cs_b = spool.tile([16, P], dt.float16, tag="cs_b")
                nc.vector.tensor_copy(cs_b[:tpc],
                                      cs[:tpc].to_broadcast([tpc, P]))
                ppos = psum.tile([P, 16], dt.float32, tag="pc", bufs=1,
                                 name=f"ppos{c}")
                nc.tensor.matmul(ppos[:, :tpc], lhsT=lst[:], rhs=mask_ch,
                                 start=True, stop=False)
                nc.tensor.matmul(ppos[:, :tpc], lhsT=cs_b[:tpc],
                                 rhs=ust[:tpc, :tpc],
                                 start=False, stop=True)
                t1 = spool.tile([P, 16], dt.float32, tag="t1")
                nc.vector.tensor_scalar_add(t1[:, :tpc], ppos[:, :tpc],
                                            -float(cap))
                t2 = spool.tile([P, 16], dt.float32, tag="t2")
                nc.vector.tensor_mul(t2[:, :tpc], t1[:, :tpc], mask_ch)
                pos_eff = chpool.tile([P, 16], dt.float32, tag="pos_eff",
                                      name=f"pos_eff{c}")
                nc.vector.tensor_scalar_add(pos_eff[:, :tpc], t2[:, :tpc],
                                            float(cap))

                pcmp = psum.tile([P, 3 * 3], dt.float32, tag="pc",
                                 bufs=1, name=f"pcmp{c}")
                for f in range(tpc):
                    perm = ppool.tile([P, CAPMAX], dt.float16, tag="perm",
                                      name=f"perm{c}_{f}")
                    nc.vector.tensor_tensor(
                        perm[:, :cap],
                        pos_eff[:, f:f + 1].to_broadcast([P, cap]),
                        slotsb[:, :cap], op=OP.is_equal)
                    rhs3 = spool.tile([P, 3], dt.float16, tag="rhs3")
                    nc.vector.tensor_scalar_add(rhs3[:, 0:1], iotasb[:],
                                                float(f * P))
                    nc.vector.tensor_copy(rhs3[:, 1:2], coef_ch[:, f:f + 1])
                    nc.vector.memset(rhs3[:, 2:3], 1.0)
                    for sg in range(nsg):
                        nc.tensor.matmul(
                            pcmp[:, 3 * sg:3 * sg + 3],
                            lhsT=perm[:, sgo[sg]:sgo[sg] + P],
                            rhs=rhs3[:],
                            start=(f == 0 and sg == 0),
                            stop=(f == tpc - 1 and sg == nsg - 1))

                idx_g_i = chpool.tile([P, 3], dt.int32, tag="idx_g",
                                      name=f"idxg{c}")
                idx_s_i = chpool.tile([P, 3], dt.int32, tag="idx_s",
                                      name=f"idxs{c}")
                coef_sg = chpool.tile([P, 3], dt.float32, tag="coef_sg",
                                      name=f"coefsg{c}")
                for sg in range(nsg):
                    cmp = spool.tile([P, 3], dt.float32, tag="cmp")
                    nc.vector.tensor_copy(cmp[:], pcmp[:, 3 * sg:3 * sg + 3])
                    nc.vector.tensor_copy(coef_sg[:, sg:sg + 1], cmp[:, 1:2])
                    gidx = spool.tile([P, 1], dt.float32, tag="gidx")
                    nc.vector.tensor_scalar_add(gidx[:], cmp[:, 0:1],
                                                float(tk0))
                    nc.vector.tensor_copy(idx_g_i[:, sg:sg + 1], gidx[:])
                    iv = spool.tile([P, 1], dt.float32, tag="iv")
                    nc.vector.tensor_scalar(iv[:], cmp[:, 2:3],
                                            -float(ntok), float(ntok),
                                            op0=OP.mult, op1=OP.add)
                    sidx = spool.tile([P, 1], dt.float32, tag="sidx")
                    nc.vector.tensor_add(sidx[:], cmp[:, 0:1], iv[:])
                    nc.vector.tensor_copy(idx_s_i[:, sg:sg + 1], sidx[:])
                return idx_g_i, idx_s_i, coef_sg

            def gather(c, idx_g_i):
                """indirect-gather selected bf16 rows + PE transpose."""
                tk0, ntok, cap, sgo = CHUNK_SPECS[c]
                nsg = len(sgo)
                xgT = xgtpool.tile([P, KD, CAPMAX], dt.bfloat16, tag="xgT",
                                   name=f"xgT{c}")
                xgs = []
                for sg in range(nsg):
                    xg = xgpool.tile([P, D], dt.bfloat16, tag="xg", bufs=3,
                                     name=f"xg{c}_{sg}")
                    nc.gpsimd.indirect_dma_start(
                        out=xg[:], out_offset=None, in_=x_d[:, :],
                        in_offset=bass.IndirectOffsetOnAxis(
                            ap=idx_g_i[:, sg:sg + 1], axis=0))
                    xgs.append(xg)
                for sg in range(nsg):
                    for g in range(KD // 2):
                        pt2 = psum.tile([P, 2, P], dt.bfloat16, tag="pa",
                                        bufs=2, name=f"pt2_{c}_{sg}_{g}")
                        for j in range(2):
                            kc = 2 * g + j
                            nc.tensor.matmul(
                                pt2[:, j, :],
                                lhsT=xgs[sg][:, kc * P:(kc + 1) * P],
                                rhs=identb[:], is_transpose=True,
                                start=(j == 0), stop=(j == 1))
                        nc.vector.tensor_copy(
                            xgT[:, 2 * g:2 * g + 2, sgo[sg]:sgo[sg] + P],
                            pt2[:])
                return xgT

            def zero_partial(c):
                tk0, ntok, cap, sgo = CHUNK_SPECS[c]
                for i in range(ntok // P):
                    nc.sync.dma_start(
                        partials[c][i * P:(i + 1) * P, :], zt[:])

            def ffn_w1(c, xgT):
                tk0, ntok, cap, sgo = CHUNK_SPECS[c]
                hT = hpool.tile([P, NDJ, CAPMAX], dt.bfloat16, tag="hT",
                                name=f"hT{c}")
                for dj in range(NDJ):
                    ph = psum.tile([P, 320], dt.float32, tag="ph", bufs=2,
                                   name=f"ph{c}_{dj}")
                    for kc in range(KD):
                        nc.tensor.matmul(
                            ph[:, :cap],
                            lhsT=w1sb[:, dj, kc, :],
                            rhs=xgT[:, kc, :cap],
                            start=(kc == 0), stop=(kc == KD - 1))
                    nc.scalar.activation(hT[:, dj, :cap], ph[:, :cap],
                                         AF.Relu, bias=b1sb[:, dj:dj + 1])
                return hT

            def ffn_w2(c, hT, idx_s_i, coef_sg):
                tk0, ntok, cap, sgo = CHUNK_SPECS[c]
                nsg = len(sgo)
                youts = [ypool.tile([P, D], dt.bfloat16, tag="yout",
                                    name=f"yout{c}_{i}") for i in range(nsg)]
                for sg in range(nsg):
                    for h in range(2):
                        hs = slice(h * HALF, (h + 1) * HALF)
                        py = psum.tile([P, HALF], dt.float32, tag="pys",
                                       bufs=3, name=f"py{c}_{sg}_{h}")
                        for dj in range(NDJ):
                            nc.tensor.matmul(
                                py[:],
                                lhsT=hT[:, dj, sgo[sg]:sgo[sg] + P],
                                rhs=w2sb[:, dj, hs],
                                start=(dj == 0), stop=(dj == NDJ - 1))
                        ytmp = spool.tile([P, HALF], dt.float32, tag="ytmp",
                                          bufs=1)
                        nc.vector.tensor_add(ytmp[:], py[:], b2b[:, hs])
                        nc.vector.tensor_scalar_mul(youts[sg][:, hs],
                                                    ytmp[:],
                                                    coef_sg[:, sg:sg + 1])
                for sg in range(nsg):
                    nc.gpsimd.indirect_dma_start(
                        out=partials[c][:, :],
                        out_offset=bass.IndirectOffsetOnAxis(
                            ap=idx_s_i[:, sg:sg + 1], axis=0),
                        in_=youts[sg][:], in_offset=None)
                nc.gpsimd.collective_compute(
                    "ReduceScatter", mybir.AluOpType.add, replica_groups=rg,
                    ins=[partials[c][0:ntok, :].opt()],
                    outs=[rs_outs[c][:, :].opt()])

            def phase1_dp(c):
                """slab load + mask/coef + compact + gather for chunk c."""
                tk0, ntok, cap, sgo = CHUNK_SPECS[c]
                tpc = ntok // P
                slab = spool.tile([P, 8, E], dt.float32, tag="slab",
                                  name=f"slab{c}", bufs=2)
                f0 = (tk0 - 1024) // P
                # gpsimd queue: ordered behind the AllGather trigger, so
                # the collective-completion wait is enforced by queue order
                # (cross-queue sem waits on collectives proved racy).
                for f in range(tpc):
                    nc.gpsimd.dma_start(slab[:, f, :], gateall[f0 + f, :, :])
                selb = selsb[:, None, :].to_broadcast([P, tpc, E])
                tmp = spool.tile([P, 8, E], dt.float32, tag="tmpsl")
                nc.vector.tensor_mul(tmp[:, :tpc, :], slab[:, :tpc, :], selb)
                coef_ch = chpool.tile([P, 8], dt.float32, tag="coef",
                                      name=f"coef{c}")
                nc.vector.reduce_sum(coef_ch[:, :tpc], tmp[:, :tpc, :],
                                     axis=mybir.AxisListType.X)
                mask_ch = chpool.tile([P, 8], dt.float16, tag="mask",
                                      name=f"mask{c}")
                nc.vector.tensor_scalar(mask_ch[:, :tpc], coef_ch[:, :tpc],
                                        0.0, None, op0=OP.is_gt)
                idx_g_i, idx_s_i, coef_sg = compact(c, mask_ch, coef_ch)
                zero_partial(c)
                xgT = gather(c, idx_g_i)
                return xgT, idx_s_i, coef_sg

            # ============ emission ============
            # --- DP gate slice first (AG triggers early, completes in the
            # shadow of chunk-0's replicated gate + FFN) ---
            logitD = gate(1024, DPTOK // P, "dp")
            coefE = softmax_all(logitD, DPTOK // P, "dp")
            for f in range(DPTOK // P):
                nc.scalar.dma_start(gatebuf[f, :, :], coefE[:, f, :])
            nc.gpsimd.collective_compute(
                "AllGather", mybir.AluOpType.bypass, replica_groups=rg,
                ins=[gatebuf[:, :, :].opt()],
                outs=[gateall[:, :, :].opt()])

            # --- chunk 0: replicated gate + compact + gather ---
            logit0 = gate(0, 8, "c0")
            late_consts()
            mask0, coef0 = softmax_own(logit0, 8, "c0")
            idx_g0, idx_s0, coef_sg0 = compact(0, mask0, coef0)
            zero_partial(0)
            xgT0 = gather(0, idx_g0)

            # --- pipelined FFN ---
            prev = (xgT0, idx_s0, coef_sg0)
            for c in range(NCHUNK):
                xgT, idx_s_i, coef_sg = prev
                hT = ffn_w1(c, xgT)
                # phase-1 of c+1 before ffn_w2(c): its gpsimd ops (slab,
                # gathers) must precede scatters(c)+RS(c) in queue order,
                # or the chunk boundary serializes on the RS trigger.
                if c + 1 < NCHUNK:
                    prev = phase1_dp(c + 1)
                ffn_w2(c, hT, idx_s_i, coef_sg)

            # out copies (earlier RS long done; last copy is the tail)
            for c, (tk0, ntok, cap, sgo) in enumerate(CHUNK_SPECS):
                nrr = ntok // N_CORES
                nc.gpsimd.dma_start(
                    out_d[OUT_OFFS[c]:OUT_OFFS[c] + nrr, :], rs_outs[c][:, :])

    nc.compile()
    return nc


def _host_inputs(x, W1, b1, W2, b2, Wg, bg):
    bf16 = ml_dtypes.bfloat16
    f32 = np.float32
    f16 = np.float16
    x2 = np.ascontiguousarray(x.reshape(N, D), dtype=f32)
    xb = x2.astype(bf16)
    xt = np.ascontiguousarray(x2.T)           # [D, N]
    xtt = xt.reshape(KD, P, N)                # [kc, p, tok]
    lst = np.triu(np.ones((P, P), f16), k=1)
    ust = np.triu(np.ones((16, 16), f16), k=1)
    slot = np.tile(np.arange(CAPMAX, dtype=f32), (P, 1))
    iota = np.arange(P, dtype=f32).reshape(P, 1)
    wgt = np.ascontiguousarray(
        Wg.reshape(KD, P, E).transpose(1, 0, 2))          # [P, KD, E] f32
    wg_hi = wgt.astype(bf16)
    wg_lo = (wgt - wg_hi.astype(f32)).astype(bf16)
    wg = np.stack([wg_hi, wg_lo], axis=2)                 # [P, KD, 2, E]
    in_maps = []
    for e in range(N_CORES):
        sel = np.zeros((E,), f32)
        sel[e] = 1.0
        xtgf = np.empty((P, KD, 1024 + DPTOK), f32)
        xtgf[:, :, :1024] = xtt[:, :, :1024].transpose(1, 0, 2)
        t0 = 1024 + e * DPTOK
        xtgf[:, :, 1024:] = xtt[:, :, t0:t0 + DPTOK].transpose(1, 0, 2)
        xtg_hi = xtgf.astype(bf16)
        xtg_lo = (xtgf - xtg_hi.astype(f32)).astype(bf16)
        xtg = np.stack([xtg_hi, xtg_lo], axis=2)    # [P, KD, 2, cols]
        in_maps.append({
            "x": xb,
            "xtg": xtg,
            "w1": np.ascontiguousarray(
                W1[e].reshape(KD, P, NDJ, P).transpose(1, 2, 0, 3)
            ).astype(bf16),
            "w2": np.ascontiguousarray(
                W2[e].reshape(NDJ, P, D).transpose(1, 0, 2)).astype(bf16),
            "b1": np.ascontiguousarray(
                b1[e].reshape(NDJ, P).T).astype(f32),
            "b2": np.tile(b2[e].astype(bf16), (P, 1)),
            "wg": wg,
            "bg": np.tile(bg.astype(f32), (P, 1)),
            "sel": np.tile(sel, (P, 1)),
            "lst": lst, "ust": ust, "slot": slot, "iota": iota,
        })
    return in_maps


def _assemble(results):
    out = np.empty((N, D), np.float32)
    for r in range(N_CORES):
        shard = np.asarray(results[r]["out_shard"]).reshape(
            OROWS, D).astype(np.float32)
        for c, (tk0, ntok, cap, sgo) in enumerate(CHUNK_SPECS):
            nrr = ntok // N_CORES
            t0 = tk0 + r * nrr
            out[t0:t0 + nrr, :] = shard[OUT_OFFS[c]:OUT_OFFS[c] + nrr]
    return out.reshape(B, L, D)


def kernel(x, W1, b1, W2, b2, Wg, bg, k):
    from concourse.bass_utils import run_bass_kernel_spmd

    assert int(k) == 2
    if "nc" not in _cache:
        _cache["nc"] = _build()
    nc = _cache["nc"]
    in_maps = _host_inputs(np.asarray(x), np.asarray(W1), np.asarray(b1),
                           np.asarray(W2), np.asarray(b2), np.asarray(Wg),
                           np.asarray(bg))
    res = run_bass_kernel_spmd(nc, in_maps, core_ids=list(range(N_CORES)),
                               **_cache.get("run_kwargs", {}))
    _cache["last_result"] = res
    return _assemble(res.results)
